# revision 1
# baseline (speedup 1.0000x reference)
"""Causal self-attention (B=2, S=2048, D=1024, H=16) on 8 TRN2 NeuronCores.

Sharding: batch (2) x head-group (4 heads each) -> 8 cores. Each core computes
Q/K/V projections for its 4 heads, causal flash-attention, and a partial
output projection (its 256 columns of the concatenated head outputs against
the matching rows of Wo^T). Host sums the 4 partials per batch and adds the
bias terms (bv @ Wo.T + bo), which are x-independent.

All large inputs are packed host-side into ONE [128, 24576] f32 tensor laid
out so every partition's data is contiguous in DRAM (128 big DMA descriptors
instead of thousands of small ones). Column map per partition p:
  [     0:16384)  xT   tiles: xt[p, c, s] = x[b].T[128c+p, s]   (8 x 2048)
  [16384:18432)  wqT  tiles: wq[p, c, d] = Wq.T[:, sl][128c+p, d] (8 x 256)
  [18432:20480)  wkT  same for Wk
  [20480:22528)  wvT  same for Wv
  [22528:24576)  woT  tiles: wo[p, t, e] = Wo.T[sl, :][128t+p, e] (2 x 1024)
Biases travel in a tiny [128, 4] side tensor (bq | bk halves).

Compute per core (all matmuls at the fast 1 col/cycle rate):
  - projections in f32r; QT/KT evacuated to fp16 with bias fused (DVE)
  - scores^T[k,q] tiles via fp16 matmuls, 2 heads row-packed per 128
    partitions (concurrent in the PE array)
  - exp on ACT with the 1/sqrt(dk) scale fused, fp16 out
  - causal masking: multiplicative 0/1 fp16 mask on the 4 straddle shapes
  - PV matmul fp16 with a ones column appended to V so the softmax
    denominator falls out of the same matmul (psum row 64)
  - 1/denom via reciprocal_approx_fast + gpsimd partition_broadcast
  - out-projection f32r against Wo^T rows; one output DMA per 512-row chunk

The attention inner loop is ACT(exp)-throughput-bound while projections are
PE-bound, and the PE executes its queue strictly in order — so projection
and out-projection matmuls are emitted as a generator of small quanta that
the attention k-loop drains between its own matmuls. That pads the
scores->exp->PV dependency gaps with useful PE work instead of stalls.
Per-s-chunk SBUF tiles (not one big tensor) keep the cross-phase
dependencies precise.
"""

import numpy as np

N_CORES = 8
B, S, D = 2, 2048, 1024
H_PER_CORE = 4
DSL = 256
NC_TILES = 8
SCH = 512
NSCH = S // SCH
NST = S // 128

XT_O = 0
WQ_O = 16384
WK_O = WQ_O + 2048
WV_O = WK_O + 2048
WO_O = WV_O + 2048
IN_COLS = WO_O + 2048  # 24576

_cache = {}


def _build(reps=1, dma="loop", drain=(1, 1, 1, 1), pools=(2, 3, 1, 2), ep_bufs=8):
    import contextlib
    import concourse.mybir as mybir
    import concourse.tile as tile
    from concourse import bacc

    f32 = mybir.dt.float32
    f32r = mybir.dt.float32r
    f16 = mybir.dt.float16
    EXP = mybir.ActivationFunctionType.Exp

    nc = bacc.Bacc("TRN2", target_bir_lowering=False, debug=False,
                   num_devices=N_CORES)

    big = nc.dram_tensor("big", [128, IN_COLS], f32r, kind="ExternalInput").ap()
    bqk = nc.dram_tensor("bqk", [128, 4], f32, kind="ExternalInput").ap()
    y = nc.dram_tensor("y", [S, D], f32, kind="ExternalOutput").ap()

    with tile.TileContext(nc) as tc:
        with contextlib.ExitStack() as ctx:
            singles = ctx.enter_context(tc.tile_pool(name="singles", bufs=1))
            work = ctx.enter_context(tc.tile_pool(name="work", bufs=1))

            big_sb = singles.tile([128, IN_COLS], f32r)
            xt_sb = big_sb[:, XT_O:WQ_O].rearrange("p (c s) -> p c s", c=NC_TILES)
            wq_sb = big_sb[:, WQ_O:WK_O].rearrange("p (c d) -> p c d", c=NC_TILES)
            wk_sb = big_sb[:, WK_O:WV_O].rearrange("p (c d) -> p c d", c=NC_TILES)
            wv_sb = big_sb[:, WV_O:WO_O].rearrange("p (c d) -> p c d", c=NC_TILES)
            wo_sb = big_sb[:, WO_O:IN_COLS].rearrange("p (t e) -> p t e", t=2)
            bqk_sb = singles.tile([128, 4], f32)

            # per-s-chunk tiles -> precise cross-phase dependencies
            qt_sb = [work.tile([128, 2, SCH], f16, name=f"qt{j}", tag=f"qt{j}")
                     for j in range(NSCH)]
            kt_sb = [work.tile([128, 2, SCH], f16, name=f"kt{j}", tag=f"kt{j}")
                     for j in range(NSCH)]
            v_sb = [work.tile([128, 4, 260], f16, name=f"v{j}", tag=f"v{j}")
                    for j in range(NSCH)]
            att_sb = [[work.tile([128, SCH], f32r, name=f"att{j}_{p}", tag=f"att{j}_{p}")
                       for p in range(2)] for j in range(NSCH)]
            masks = [singles.tile([128, SCH], f16, name=f"mask{m}", tag=f"mask{m}")
                     for m in range(4)]

            # causal 0/1 masks: block row k (partition), col q;
            # valid iff q - k - 128*m >= 0
            for m in range(4):
                nc.gpsimd.memset(masks[m], 1.0)
                nc.gpsimd.affine_select(
                    out=masks[m], in_=masks[m],
                    compare_op=mybir.AluOpType.is_ge, fill=0.0,
                    base=-128 * m, pattern=[[1, SCH]], channel_multiplier=-1)
            # ones columns of V (col 64 of each head slot), written once:
            # per-rep V copies only touch cols 0..63 of each slot.
            for j in range(NSCH):
                nc.gpsimd.memset(v_sb[j], 1.0)

            def dma_in():
                # weights + biases first (first matmuls need them), then x
                # per c-tile so projections start as soon as chunk 0 lands.
                nc.scalar.dma_start(out=big_sb[:, WQ_O:IN_COLS], in_=big[:, WQ_O:IN_COLS])
                nc.scalar.dma_start(out=bqk_sb, in_=bqk)
                for c in range(NC_TILES):
                    cs = slice(XT_O + 2048 * c, XT_O + 2048 * (c + 1))
                    nc.sync.dma_start(out=big_sb[:, cs], in_=big[:, cs])

            if dma == "once":
                dma_in()

            def body(_iv=None):
                with contextlib.ExitStack() as bctx:
                    if dma == "loop":
                        dma_in()

                    pp = bctx.enter_context(tc.tile_pool(name="pp", bufs=pools[0], space="PSUM"))
                    sp_ = bctx.enter_context(tc.tile_pool(name="sp", bufs=pools[1], space="PSUM"))
                    vp = bctx.enter_context(tc.tile_pool(name="vp", bufs=pools[2], space="PSUM"))
                    op_ = bctx.enter_context(tc.tile_pool(name="op", bufs=pools[3], space="PSUM"))
                    ep = bctx.enter_context(tc.tile_pool(name="ep", bufs=ep_bufs))
                    bp = bctx.enter_context(tc.tile_pool(name="bp", bufs=6))
                    yo = bctx.enter_context(tc.tile_pool(name="yo", bufs=2))

                    def proj_gen(sc):
                        """Projection work for s-chunk sc as small PE quanta."""
                        scs = slice(SCH * sc, SCH * (sc + 1))
                        for w_sb, dst, boff in ((wq_sb, qt_sb[sc], 0),
                                                (wk_sb, kt_sb[sc], 2)):
                            ps = [pp.tile([128, SCH], f32, name=f"pj{h}", tag="qk")
                                  for h in range(2)]
                            for c in range(NC_TILES):
                                for half in range(2):
                                    nc.tensor.matmul(
                                        ps[half], lhsT=w_sb[:, c, 128 * half:128 * (half + 1)],
                                        rhs=xt_sb[:, c, scs],
                                        start=(c == 0), stop=(c == NC_TILES - 1))
                                yield
                            for half in range(2):
                                nc.vector.tensor_scalar_add(
                                    dst[:, half, :], ps[half],
                                    bqk_sb[:, boff + half:boff + half + 1])
                            yield
                        for t4 in range(4):
                            t = 4 * sc + t4
                            v_ps = vp.tile([128, DSL], f32, name="vps", tag="v")
                            for c in range(NC_TILES):
                                nc.tensor.matmul(
                                    v_ps, lhsT=xt_sb[:, c, 128 * t:128 * (t + 1)],
                                    rhs=wv_sb[:, c, :], start=(c == 0),
                                    stop=(c == NC_TILES - 1))
                                if c % 2:
                                    yield
                            nc.any.tensor_copy(
                                out=v_sb[sc].rearrange("p t (h e) -> p t h e", h=4)[:, t4, :, 0:64],
                                in_=v_ps.rearrange("p (h e) -> p h e", h=4))
                            yield

                    def outp_gen(j):
                        """Out-projection for q-chunk j as small PE quanta."""
                        y_sb = yo.tile([128, 4, D], f32, name="ysb", tag="ysb")
                        for t4 in range(4):
                            for e in range(2):
                                es = slice(512 * e, 512 * (e + 1))
                                y_ps = vp.tile([128, 512], f32, name="yps", tag="v")
                                for pair in range(2):
                                    nc.tensor.matmul(
                                        y_ps, lhsT=att_sb[j][pair][:, 128 * t4:128 * (t4 + 1)],
                                        rhs=wo_sb[:, pair, es],
                                        start=(pair == 0), stop=(pair == 1))
                                nc.any.tensor_copy(out=y_sb[:, t4, es], in_=y_ps)
                                yield
                        eng = nc.sync if j % 2 == 0 else nc.scalar
                        eng.dma_start(
                            out=y[SCH * j:SCH * (j + 1), :].rearrange("(t p) e -> p t e", p=128),
                            in_=y_sb)
                        yield

                    def attn(j, bg):
                        T = 4 * (j + 1)
                        nd = drain[j]
                        for pair in range(2):
                            o_ps = [op_.tile([65, SCH], f32, name=f"ops{h}", tag="o")
                                    for h in range(2)]
                            prev = None

                            def emit_pv(exps, t, c0):
                                cs_ = slice(c0, SCH)
                                for h in range(2):
                                    hl = 2 * pair + h
                                    nc.tensor.matmul(
                                        o_ps[h][:, cs_], lhsT=v_sb[t // 4][:, t % 4, 65 * hl:65 * hl + 65],
                                        rhs=exps[h][:, cs_], start=(t == 0), stop=(t == T - 1))

                            for t in range(T):
                                m = t - 4 * j
                                # straddle tile m: columns < 128m are fully
                                # masked -> skip them in scores/exp/mask/PV
                                c0 = 128 * m if m > 0 else 0
                                cs_ = slice(c0, SCH)
                                s_ps = [sp_.tile([128, SCH], f32, name=f"sps{h}", tag="s")
                                        for h in range(2)]
                                for h in range(2):
                                    hp = slice(64 * h, 64 * (h + 1))
                                    nc.tensor.matmul(
                                        s_ps[h][:, cs_],
                                        lhsT=kt_sb[t // 4][hp, pair, 128 * (t % 4):128 * (t % 4 + 1)],
                                        rhs=qt_sb[j][hp, pair, cs_], start=True, stop=True)
                                exps = [ep.tile([128, SCH], f16, name=f"exps{h}", tag="e")
                                        for h in range(2)]
                                for h in range(2):
                                    nc.scalar.activation(out=exps[h][:, cs_], in_=s_ps[h][:, cs_],
                                                         func=EXP, scale=0.125)
                                if m >= 0:
                                    for h in range(2):
                                        nc.vector.tensor_mul(exps[h][:, cs_], exps[h][:, cs_],
                                                             masks[m][:, cs_])
                                for _ in range(nd):
                                    next(bg, None)
                                if prev is not None:
                                    emit_pv(*prev)
                                prev = (exps, t, c0)
                            emit_pv(*prev)

                            # normalize: att = O[0:64] * bcast(1/denom);
                            # interleave the two heads' chains
                            bcs = []
                            for h in range(2):
                                bc = bp.tile([128, SCH], f32, name=f"bc{h}", tag="bc")
                                nc.vector.tensor_copy(out=bc[0:1, :], in_=o_ps[h][64:65, :])
                                bcs.append(bc)
                            for h in range(2):
                                nc.vector.reciprocal_approx_fast(
                                    out=bcs[h][0:1, :], in_=bcs[h][0:1, :])
                            for h in range(2):
                                nc.gpsimd.partition_broadcast(
                                    out_ap=bcs[h][0:64, :], in_ap=bcs[h][0:1, :])
                            for h in range(2):
                                nc.vector.tensor_mul(
                                    att_sb[j][pair][64 * h:64 * (h + 1), :],
                                    o_ps[h][0:64, :], bcs[h][0:64, :])

                    def drain_all(bg):
                        for _ in bg:
                            pass

                    def chain(*gens):
                        for g in gens:
                            yield from g

                    # proj(0) up front; then attention chunk j drains
                    # proj(j+1) and outp(j-1) quanta between its k-tiles.
                    drain_all(proj_gen(0))
                    for j in range(NSCH):
                        gens = []
                        if j > 0:
                            gens.append(outp_gen(j - 1))
                        if j + 1 < NSCH:
                            gens.append(proj_gen(j + 1))
                        bg = chain(*gens)
                        attn(j, bg)
                        drain_all(bg)
                    drain_all(outp_gen(NSCH - 1))

            if reps == 1:
                body()
            else:
                with tc.For_i(0, reps, 1):
                    body()

    nc.compile()
    return nc


def _get_nc(reps=1, **kw):
    key = (reps, tuple(sorted(kw.items())))
    if key not in _cache:
        _cache[key] = _build(reps, **kw)
    return _cache[key]


def _tiles(a, nt):
    # [nt*128, w] -> [128, nt*w] with [p, t*w:t*w+w] = a[128t+p, :]
    w = a.shape[1]
    return a.reshape(nt, 128, w).transpose(1, 0, 2).reshape(128, nt * w)


def make_in_maps(x, Wq, bq, Wk, bk, Wv, bv, Wo, bo):
    """Shard full inputs into 8 per-core input dicts."""
    in_maps = []
    for core in range(N_CORES):
        b, g = core // 4, core % 4
        sl = slice(DSL * g, DSL * (g + 1))
        big = np.concatenate([
            _tiles(np.ascontiguousarray(x[b].T), 8),
            _tiles(np.ascontiguousarray(Wq[sl, :].T), 8),
            _tiles(np.ascontiguousarray(Wk[sl, :].T), 8),
            _tiles(np.ascontiguousarray(Wv[sl, :].T), 8),
            _tiles(np.ascontiguousarray(Wo[:, sl].T), 2),
        ], axis=1)
        bqk = np.concatenate([bq[sl].reshape(2, 128).T, bk[sl].reshape(2, 128).T],
                             axis=1)
        in_maps.append({"big": big, "bqk": np.ascontiguousarray(bqk)})
    return in_maps


def kernel(x, Wq, bq, Wk, bk, Wv, bv, Wo, bo):
    from concourse.bass_utils import run_bass_kernel_spmd

    x = np.asarray(x, dtype=np.float32)
    Wq, bq = np.asarray(Wq, np.float32), np.asarray(bq, np.float32)
    Wk, bk = np.asarray(Wk, np.float32), np.asarray(bk, np.float32)
    Wv, bv = np.asarray(Wv, np.float32), np.asarray(bv, np.float32)
    Wo, bo = np.asarray(Wo, np.float32), np.asarray(bo, np.float32)

    nc = _get_nc()
    in_maps = make_in_maps(x, Wq, bq, Wk, bk, Wv, bv, Wo, bo)
    res = run_bass_kernel_spmd(nc, in_maps, core_ids=list(range(N_CORES)))

    cvec = bv @ Wo.T + bo  # x-independent bias contribution
    out = np.zeros((B, S, D), dtype=np.float32)
    for core in range(N_CORES):
        out[core // 4] += res.results[core]["y"]
    out += cvec[None, None, :]
    return out



# revision 11
# speedup vs baseline: 41.6117x; 41.6117x over previous
"""Causal self-attention (B=2, S=2048, D=1024, H=16) on 8 TRN2 NeuronCores.

Sharding: batch (2) x head-group (4 heads each) -> 8 cores. Each core computes
Q/K/V projections for its 4 heads, causal flash-attention, and a partial
output projection (its 256 columns of the concatenated head outputs against
the matching rows of Wo^T). Host sums the 4 partials per batch and adds the
bias terms (bv @ Wo.T + bo), which are x-independent.

All large inputs are packed host-side into ONE [128, 24576] f32 tensor laid
out so every partition's data is contiguous in DRAM (128 big DMA descriptors
instead of thousands of small ones). Column map per partition p:
  [     0:16384)  xT   tiles: xt[p, c, s] = x[b].T[128c+p, s]   (8 x 2048)
  [16384:18432)  wqT  tiles: wq[p, c, d] = Wq.T[:, sl][128c+p, d] (8 x 256)
  [18432:20480)  wkT  same for Wk
  [20480:22528)  wvT  same for Wv
  [22528:24576)  woT  tiles: wo[p, t, e] = Wo.T[sl, :][128t+p, e] (2 x 1024)
Biases travel in a tiny [128, 4] side tensor (bq | bk halves).

Compute per core (all matmuls at the fast 1 col/cycle rate):
  - projections in f32r; QT/KT evacuated to fp16 with bias fused (DVE)
  - scores^T[k,q] tiles via fp16 matmuls, 2 heads row-packed per 128
    partitions (concurrent in the PE array)
  - exp on ACT with the 1/sqrt(dk) scale fused, fp16 out
  - causal masking: multiplicative 0/1 fp16 mask on the 4 straddle shapes
  - PV matmul fp16 with a ones column appended to V so the softmax
    denominator falls out of the same matmul (psum row 64)
  - 1/denom via reciprocal_approx_fast + gpsimd partition_broadcast
  - out-projection f32r against Wo^T rows; one output DMA per 512-row chunk

The attention inner loop is ACT(exp)-throughput-bound while projections are
PE-bound, and the PE executes its queue strictly in order — so projection
and out-projection matmuls are emitted as a generator of small quanta that
the attention k-loop drains between its own matmuls. That pads the
scores->exp->PV dependency gaps with useful PE work instead of stalls.
Per-s-chunk SBUF tiles (not one big tensor) keep the cross-phase
dependencies precise.
"""

import numpy as np

N_CORES = 8
B, S, D = 2, 2048, 1024
H_PER_CORE = 4
DSL = 256
NC_TILES = 8
SCH = 512
NSCH = S // SCH
NST = S // 128

XT_O = 0
WQ_O = 16384
WK_O = WQ_O + 2048
WV_O = WK_O + 2048
WO_O = WV_O + 2048
IN_COLS = WO_O + 2048  # 24576

_cache = {}


def _build(reps=1, dma="loop", drain=(1, 1, 1, 1), pools=(1, 2, 1, 2), ep_bufs=4):
    import contextlib
    import concourse.mybir as mybir
    import concourse.tile as tile
    from concourse import bacc

    f32 = mybir.dt.float32
    f32r = mybir.dt.float32r
    f16 = mybir.dt.float16
    EXP = mybir.ActivationFunctionType.Exp

    nc = bacc.Bacc("TRN2", target_bir_lowering=False, debug=False,
                   num_devices=N_CORES)

    big = nc.dram_tensor("big", [128, IN_COLS], f16, kind="ExternalInput").ap()
    bqk = nc.dram_tensor("bqk", [128, 4], f32, kind="ExternalInput").ap()
    y = nc.dram_tensor("y", [S, D], f16, kind="ExternalOutput").ap()

    with tile.TileContext(nc) as tc:
        with contextlib.ExitStack() as ctx:
            singles = ctx.enter_context(tc.tile_pool(name="singles", bufs=1))
            work = ctx.enter_context(tc.tile_pool(name="work", bufs=1))

            big_sb = singles.tile([128, IN_COLS], f16)
            xt_sb = big_sb[:, XT_O:WQ_O].rearrange("p (c s) -> p c s", c=NC_TILES)
            wq_sb = big_sb[:, WQ_O:WK_O].rearrange("p (c d) -> p c d", c=NC_TILES)
            wk_sb = big_sb[:, WK_O:WV_O].rearrange("p (c d) -> p c d", c=NC_TILES)
            wv_sb = big_sb[:, WV_O:WO_O].rearrange("p (c d) -> p c d", c=NC_TILES)
            wo_sb = big_sb[:, WO_O:IN_COLS].rearrange("p (t e) -> p t e", t=2)
            bqk_sb = singles.tile([128, 4], f32)

            # per-s-chunk tiles -> precise cross-phase dependencies
            qt_sb = [work.tile([128, 2, SCH], f16, name=f"qt{j}", tag=f"qt{j}")
                     for j in range(NSCH)]
            kt_sb = [work.tile([128, 2, SCH], f16, name=f"kt{j}", tag=f"kt{j}")
                     for j in range(NSCH)]
            v_sb = [work.tile([128, 4, 260], f16, name=f"v{j}", tag=f"v{j}")
                    for j in range(NSCH)]
            att_sb = [[work.tile([128, SCH], f16, name=f"att{j}_{p}", tag=f"att{j}_{p}")
                       for p in range(2)] for j in range(NSCH)]
            masks = [singles.tile([128, 2, SCH], f16, name=f"mask{m}", tag=f"mask{m}")
                     for m in range(4)]

            # causal 0/1 masks: block row k (partition), col q;
            # valid iff q - k - 128*m >= 0 (duplicated for the 2 packed heads)
            for m in range(4):
                nc.gpsimd.memset(masks[m], 1.0)
                for h in range(2):
                    nc.gpsimd.affine_select(
                        out=masks[m][:, h, :], in_=masks[m][:, h, :],
                        compare_op=mybir.AluOpType.is_ge, fill=0.0,
                        base=-128 * m, pattern=[[1, SCH]], channel_multiplier=-1)
            # ones columns of V (col 64 of each head slot), written once:
            # per-rep V copies only touch cols 0..63 of each slot.
            for j in range(NSCH):
                nc.gpsimd.memset(v_sb[j], 1.0)

            def dma_in():
                # weights + biases first (first matmuls need them), then x
                # per c-tile so projections start as soon as chunk 0 lands.
                nc.scalar.dma_start(out=big_sb[:, WQ_O:IN_COLS], in_=big[:, WQ_O:IN_COLS])
                nc.scalar.dma_start(out=bqk_sb, in_=bqk)
                for c in range(NC_TILES):
                    cs = slice(XT_O + 2048 * c, XT_O + 2048 * (c + 1))
                    nc.sync.dma_start(out=big_sb[:, cs], in_=big[:, cs])

            if dma == "once":
                dma_in()

            def body(_iv=None):
                with contextlib.ExitStack() as bctx:
                    if dma == "loop":
                        dma_in()

                    pp = bctx.enter_context(tc.tile_pool(name="pp", bufs=pools[0], space="PSUM"))
                    sp_ = bctx.enter_context(tc.tile_pool(name="sp", bufs=pools[1], space="PSUM"))
                    vp = bctx.enter_context(tc.tile_pool(name="vp", bufs=pools[2], space="PSUM"))
                    op_ = bctx.enter_context(tc.tile_pool(name="op", bufs=pools[3], space="PSUM"))
                    ep = bctx.enter_context(tc.tile_pool(name="ep", bufs=ep_bufs))
                    bp = bctx.enter_context(tc.tile_pool(name="bp", bufs=6))
                    yo = bctx.enter_context(tc.tile_pool(name="yo", bufs=2))

                    def proj_gen(sc):
                        """Projection work for s-chunk sc as small PE quanta."""
                        scs = slice(SCH * sc, SCH * (sc + 1))
                        for w_sb, dst, boff in ((wq_sb, qt_sb[sc], 0),
                                                (wk_sb, kt_sb[sc], 2)):
                            for half in range(2):
                                ps = pp.tile([128, SCH], f32, name="pj", tag="qk")
                                for c in range(NC_TILES):
                                    nc.tensor.matmul(
                                        ps, lhsT=w_sb[:, c, 128 * half:128 * (half + 1)],
                                        rhs=xt_sb[:, c, scs],
                                        start=(c == 0), stop=(c == NC_TILES - 1))
                                    if c % 2:
                                        yield
                                nc.vector.tensor_scalar_add(
                                    dst[:, half, :], ps,
                                    bqk_sb[:, boff + half:boff + half + 1])
                                yield
                        for t4 in range(4):
                            t = 4 * sc + t4
                            v_ps = vp.tile([128, DSL], f32, name="vps", tag="v")
                            for c in range(NC_TILES):
                                nc.tensor.matmul(
                                    v_ps, lhsT=xt_sb[:, c, 128 * t:128 * (t + 1)],
                                    rhs=wv_sb[:, c, :], start=(c == 0),
                                    stop=(c == NC_TILES - 1))
                                if c % 2:
                                    yield
                            nc.any.tensor_copy(
                                out=v_sb[sc].rearrange("p t (h e) -> p t h e", h=4)[:, t4, :, 0:64],
                                in_=v_ps.rearrange("p (h e) -> p h e", h=4))
                            yield

                    def outp_gen(j):
                        """Out-projection for q-chunk j as small PE quanta."""
                        y_sb = yo.tile([128, 4, D], f16, name="ysb", tag="ysb")
                        for t4 in range(4):
                            for e in range(2):
                                es = slice(512 * e, 512 * (e + 1))
                                y_ps = vp.tile([128, 512], f32, name="yps", tag="v")
                                for pair in range(2):
                                    nc.tensor.matmul(
                                        y_ps, lhsT=att_sb[j][pair][:, 128 * t4:128 * (t4 + 1)],
                                        rhs=wo_sb[:, pair, es],
                                        start=(pair == 0), stop=(pair == 1))
                                nc.any.tensor_copy(out=y_sb[:, t4, es], in_=y_ps)
                                yield
                        eng = nc.sync if j % 2 == 0 else nc.scalar
                        eng.dma_start(
                            out=y[SCH * j:SCH * (j + 1), :].rearrange("(t p) e -> p t e", p=128),
                            in_=y_sb)
                        yield

                    def attn(j, bg):
                        T = 4 * (j + 1)
                        nd = drain[j]
                        for pair in range(2):
                            o_ps = [op_.tile([65, SCH], f32, name=f"ops{h}", tag="o")
                                    for h in range(2)]
                            prev = None

                            def emit_pv(exps, t, c0):
                                cs_ = slice(c0, SCH)
                                for h in range(2):
                                    hl = 2 * pair + h
                                    nc.tensor.matmul(
                                        o_ps[h][:, cs_], lhsT=v_sb[t // 4][:, t % 4, 65 * hl:65 * hl + 65],
                                        rhs=exps[:, h, cs_], start=(t == 0), stop=(t == T - 1))

                            for t in range(T):
                                m = t - 4 * j
                                # straddle tile m: columns < 128m are fully
                                # masked -> skip them in scores/exp/mask/PV
                                c0 = 128 * m if m > 0 else 0
                                cs_ = slice(c0, SCH)
                                s_ps = sp_.tile([128, 2, SCH], f32, name="sps", tag="s")
                                for h in range(2):
                                    hp = slice(64 * h, 64 * (h + 1))
                                    nc.tensor.matmul(
                                        s_ps[:, h, cs_],
                                        lhsT=kt_sb[t // 4][hp, pair, 128 * (t % 4):128 * (t % 4 + 1)],
                                        rhs=qt_sb[j][hp, pair, cs_], start=True, stop=True)
                                exps = ep.tile([128, 2, SCH], f16, name="exps", tag="e")
                                nc.scalar.activation(out=exps[:, :, cs_], in_=s_ps[:, :, cs_],
                                                     func=EXP, scale=0.125)
                                if m >= 0:
                                    nc.vector.tensor_mul(exps[:, :, cs_], exps[:, :, cs_],
                                                         masks[m][:, :, cs_])
                                for _ in range(nd):
                                    next(bg, None)
                                if prev is not None:
                                    emit_pv(*prev)
                                prev = (exps, t, c0)
                            emit_pv(*prev)

                            # normalize: att = O[0:64] * bcast(1/denom);
                            # interleave the two heads' chains
                            bcs = []
                            for h in range(2):
                                bc = bp.tile([128, SCH], f32, name=f"bc{h}", tag="bc")
                                nc.vector.tensor_copy(out=bc[0:1, :], in_=o_ps[h][64:65, :])
                                bcs.append(bc)
                            for h in range(2):
                                nc.vector.reciprocal_approx_fast(
                                    out=bcs[h][0:1, :], in_=bcs[h][0:1, :])
                            for h in range(2):
                                nc.gpsimd.partition_broadcast(
                                    out_ap=bcs[h][0:64, :], in_ap=bcs[h][0:1, :])
                            for h in range(2):
                                nc.vector.tensor_mul(
                                    att_sb[j][pair][64 * h:64 * (h + 1), :],
                                    o_ps[h][0:64, :], bcs[h][0:64, :])

                    def drain_all(bg):
                        for _ in bg:
                            pass

                    def chain(*gens):
                        for g in gens:
                            yield from g

                    # proj(0) up front; then attention chunk j drains
                    # proj(j+1) and outp(j-1) quanta between its k-tiles.
                    drain_all(proj_gen(0))
                    for j in range(NSCH):
                        gens = []
                        if j > 0:
                            gens.append(outp_gen(j - 1))
                        if j + 1 < NSCH:
                            gens.append(proj_gen(j + 1))
                        bg = chain(*gens)
                        attn(j, bg)
                        drain_all(bg)
                    drain_all(outp_gen(NSCH - 1))

            if reps == 1:
                body()
            else:
                with tc.For_i(0, reps, 1):
                    body()

    nc.compile()
    return nc


def _get_nc(reps=1, **kw):
    key = (reps, tuple(sorted(kw.items())))
    if key not in _cache:
        _cache[key] = _build(reps, **kw)
    return _cache[key]


def _tiles(a, nt):
    # [nt*128, w] -> [128, nt*w] with [p, t*w:t*w+w] = a[128t+p, :]
    w = a.shape[1]
    return a.reshape(nt, 128, w).transpose(1, 0, 2).reshape(128, nt * w)


def make_in_maps(x, Wq, bq, Wk, bk, Wv, bv, Wo, bo):
    """Shard full inputs into 8 per-core input dicts (fp16 payload)."""
    in_maps = []
    for core in range(N_CORES):
        b, g = core // 4, core % 4
        sl = slice(DSL * g, DSL * (g + 1))
        big = np.concatenate([
            _tiles(np.ascontiguousarray(x[b].T), 8),
            _tiles(np.ascontiguousarray(Wq[sl, :].T), 8),
            _tiles(np.ascontiguousarray(Wk[sl, :].T), 8),
            _tiles(np.ascontiguousarray(Wv[sl, :].T), 8),
            _tiles(np.ascontiguousarray(Wo[:, sl].T), 2),
        ], axis=1).astype(np.float16)
        bqk = np.concatenate([bq[sl].reshape(2, 128).T, bk[sl].reshape(2, 128).T],
                             axis=1)
        in_maps.append({"big": big, "bqk": np.ascontiguousarray(bqk)})
    return in_maps


def kernel(x, Wq, bq, Wk, bk, Wv, bv, Wo, bo):
    from concourse.bass_utils import run_bass_kernel_spmd

    x = np.asarray(x, dtype=np.float32)
    Wq, bq = np.asarray(Wq, np.float32), np.asarray(bq, np.float32)
    Wk, bk = np.asarray(Wk, np.float32), np.asarray(bk, np.float32)
    Wv, bv = np.asarray(Wv, np.float32), np.asarray(bv, np.float32)
    Wo, bo = np.asarray(Wo, np.float32), np.asarray(bo, np.float32)

    nc = _get_nc()
    in_maps = make_in_maps(x, Wq, bq, Wk, bk, Wv, bv, Wo, bo)
    res = run_bass_kernel_spmd(nc, in_maps, core_ids=list(range(N_CORES)))

    cvec = bv @ Wo.T + bo  # x-independent bias contribution
    out = np.zeros((B, S, D), dtype=np.float32)
    for core in range(N_CORES):
        out[core // 4] += res.results[core]["y"].astype(np.float32)
    out += cvec[None, None, :]
    return out

